# revision 7
# baseline (speedup 1.0000x reference)
"""Trainium2 Bass kernel for nn_LNKillingRelu (fp16 pipeline).

Math (per batch b, channel g, point n; L=8 lie-algebra coords):
    d[b,g,:,n]  = sum_f W[g,f] * x[b,f,:,n]          (64x64 linear over channels)
    kf[b,g,n]   = x[b,g,:,n]^T K d[b,g,:,n]          (8x8 Killing bilinear form)
    out         = x + relu(kf) * d                    (broadcast kf over L)

K is symmetric & sparse: pairs (0<->6, 1<->7, 2<->3) plus [[2,1],[1,2]] on
coords (4,5).  We use kf = sum_l (Kx)[l] * d[l] with
(Kx) = [x6, x7, x3, x2, 2x4+x5, x4+2x5, x0, x1].

Everything on-device runs in fp16 (tolerance is 2e-2 scale-relative; fp16
keeps it ~1e-3): halves DMA bytes (the binding resource - all DMA transfers
serialize at ~360B/ns aggregate), doubles DVE throughput (2x packed mode),
and runs the PE at 1 cycle/row instead of 4.  Host converts fp32<->fp16.

Sharding: data-parallel over batch B=16 -> 2 batches per core (8 cores).
Per-core layout: partitions = (batch-pair, F=64) = 128; free = (L, n-chunk).
PE computes d with a block-diag(W^T, W^T) 128x128 stationary fp16 weight.
ACT drains PSUM->SBUF (fp32->fp16); DVE+GPSIMD share the Killing products,
tree-reduce, and the out = x + relu(kf)*d tail.

Software pipelining: relu(k-1) and the out-stage(k-1) are emitted inside
chunk k - relu goes on ACT *before* chunk k's PSUM drains (so ACT never
stalls on the DVE kf-chain), and the out-stage fills the DVE/Pool gap while
chunk k's D is still in the PE/ACT pipe.  Chunk sizes ramp 256 -> 512 -> 256
to shorten the pipeline head/tail; all tiles are allocated at max width and
column-sliced so the tile pools see a single tag.
"""

import numpy as np
from contextlib import ExitStack

import concourse.bass as bass
import concourse.bacc as bacc
import concourse.tile as tile
from concourse import mybir
from concourse.bass_utils import run_bass_kernel_spmd

B, F, L, N = 16, 64, 8, 4096
N_CORES = 8
B_PER_CORE = B // N_CORES  # 2

F16 = mybir.dt.float16
F32 = mybir.dt.float32
MULT = mybir.AluOpType.mult
ADD = mybir.AluOpType.add

NQ = 256  # psum quarter width (4 banks fp32 at L=8)


def _bcast_l(ap: bass.AP, l: int) -> bass.AP:
    """[128, NT] AP -> [128, l, NT] with a zero-stride middle dim."""
    return bass.AP(tensor=ap.tensor, offset=ap.offset,
                   ap=[ap.ap[0], [0, l], ap.ap[1]])


# default engine assignment: 'v' = DVE, 'p' = GPSIMD(Pool)
DEFAULT_ASSIGN = dict(pa='v', pb='v', pc='p', pd='v', pe='v',
                      xe0='p', xe1='p', l1='v', l2='v', l3='v',
                      osplit=3, pipe=1)
DEFAULT_CHUNKS = (256, 256, 512, 512, 512, 512, 512, 512, 256, 256)


def _build(chunks=DEFAULT_CHUNKS, assign=None):
    asn = dict(DEFAULT_ASSIGN)
    if assign:
        asn.update(assign)
    assert sum(chunks) == N
    ntmax = max(chunks)
    nc = bacc.Bacc("TRN2", target_bir_lowering=False, debug=False,
                   num_devices=N_CORES)
    x = nc.dram_tensor("x", [B_PER_CORE, F, L, N], F16, kind="ExternalInput").ap()
    w2t = nc.dram_tensor("w2t", [128, 128], F16, kind="ExternalInput").ap()
    out = nc.dram_tensor("out", [B_PER_CORE, F, L, N], F16, kind="ExternalOutput").ap()

    xv = x.rearrange("b f l n -> (b f) l n")    # [128, 8, N]
    ov = out.rearrange("b f l n -> (b f) l n")

    def eng(key):
        return nc.vector if asn[key] == 'v' else nc.gpsimd

    with ExitStack() as ctx:
        tc = ctx.enter_context(tile.TileContext(nc))
        singles = ctx.enter_context(tc.tile_pool(name="singles", bufs=1))
        xpool = ctx.enter_context(tc.tile_pool(name="xp", bufs=3))
        dpool = ctx.enter_context(tc.tile_pool(name="dp", bufs=3))
        vpool = ctx.enter_context(tc.tile_pool(name="vp", bufs=2))
        tpool = ctx.enter_context(tc.tile_pool(name="tp", bufs=2))
        mpool = ctx.enter_context(tc.tile_pool(name="mp", bufs=2))
        opool = ctx.enter_context(tc.tile_pool(name="op", bufs=3))
        rpool = ctx.enter_context(tc.tile_pool(name="rp", bufs=3))
        psum = ctx.enter_context(tc.tile_pool(name="ps", bufs=2, space="PSUM"))

        w_sb = singles.tile([128, 128], F16)
        nc.sync.dma_start(out=w_sb[:], in_=w2t)

        pending = []  # deferred tail: (X, D, kf, n0, nt)

        def emit_relu(item):
            X, D, kf, n0, nt = item
            r = rpool.tile([128, ntmax], F16, tag="r")
            nc.scalar.activation(r[:, 0:nt], kf[:, 0:nt],
                                 mybir.ActivationFunctionType.Relu)
            return (X, D, r, n0, nt)

        def emit_out_stage(item):
            X, D, r, n0, nt = item
            M = mpool.tile([128, L, ntmax], F16, tag="M")
            O = opool.tile([128, L, ntmax], F16, tag="O")
            s = asn['osplit']
            if s > 0:
                nc.vector.tensor_mul(M[:, 0:s, 0:nt], D[:, 0:s, 0:nt],
                                     _bcast_l(r[:, 0:nt], s))
                nc.vector.tensor_add(O[:, 0:s, 0:nt], X[:, 0:s, 0:nt],
                                     M[:, 0:s, 0:nt])
            if s < L:
                rb = _bcast_l(r[:, 0:nt], L - s)
                nc.gpsimd.tensor_mul(M[:, s:L, 0:nt], D[:, s:L, 0:nt], rb)
                nc.gpsimd.tensor_add(O[:, s:L, 0:nt], X[:, s:L, 0:nt],
                                     M[:, s:L, 0:nt])
            nc.sync.dma_start(out=ov[:, :, n0:n0 + nt], in_=O[:, :, 0:nt])

        n0 = 0
        for nt in chunks:
            # in-DMA split by column halves: the first half's matmuls can
            # start while the second half is still transferring.
            X = xpool.tile([128, L, ntmax], F16, tag="X")
            nh = nt // 2
            nc.sync.dma_start(out=X[:, :, 0:nh], in_=xv[:, :, n0:n0 + nh])
            nc.sync.dma_start(out=X[:, :, nh:nt], in_=xv[:, :, n0 + nh:n0 + nt])

            # Xe = (2x4+x5, x4+2x5): only needs X, fills Pool early.
            Xe = tpool.tile([128, 2, ntmax], F16, tag="Xe")
            eng('xe0').scalar_tensor_tensor(Xe[:, 0, 0:nt], in0=X[:, 4, 0:nt],
                                            scalar=2.0, in1=X[:, 5, 0:nt],
                                            op0=MULT, op1=ADD)
            eng('xe1').scalar_tensor_tensor(Xe[:, 1, 0:nt], in0=X[:, 5, 0:nt],
                                            scalar=2.0, in1=X[:, 4, 0:nt],
                                            op0=MULT, op1=ADD)

            # relu of the previous chunk: on ACT *before* this chunk's
            # copies, so it never blocks them (its input is long done).
            prev = None
            if pending:
                prev = emit_relu(pending.pop(0))

            # D = W @ X via psum quarter-tiles (4 banks each, double buffered)
            D = dpool.tile([128, L, ntmax], F16, tag="D")
            for q in range(nt // NQ):
                c0 = q * NQ
                ps = psum.tile([128, L, NQ], F32, tag="ps")
                for k in range(L // 2):
                    sl = slice(2 * k, 2 * k + 2)
                    nc.tensor.matmul(ps[:, sl, :], lhsT=w_sb[:],
                                     rhs=X[:, sl, c0:c0 + NQ],
                                     start=True, stop=True)
                nc.scalar.copy(D[:, :, c0:c0 + NQ], ps[:])

            # deferred out-stage of the previous chunk: ready to run, fills
            # the DVE/Pool gap while this chunk's D is in the PE/ACT pipe.
            if prev is not None:
                emit_out_stage(prev)

            # V[l] = (Kx)[l] * D[l]
            V = vpool.tile([128, L, ntmax], F16, tag="V")
            eng('pa').tensor_mul(V[:, 0:2, 0:nt], X[:, 6:8, 0:nt], D[:, 0:2, 0:nt])
            eng('pb').tensor_mul(V[:, 2, 0:nt], X[:, 3, 0:nt], D[:, 2, 0:nt])
            eng('pc').tensor_mul(V[:, 3, 0:nt], X[:, 2, 0:nt], D[:, 3, 0:nt])
            eng('pd').tensor_mul(V[:, 4:6, 0:nt], Xe[:, :, 0:nt], D[:, 4:6, 0:nt])
            eng('pe').tensor_mul(V[:, 6:8, 0:nt], X[:, 0:2, 0:nt], D[:, 6:8, 0:nt])

            # kf = sum_l V[l] via tree adds (tensor_tensor keeps the fp16 2x
            # DVE mode; tensor_reduce over strided l would run at full rate)
            T4 = tpool.tile([128, 4, ntmax], F16, tag="T4")
            T2 = tpool.tile([128, 2, ntmax], F16, tag="T2")
            kf = rpool.tile([128, ntmax], F16, tag="kf")
            eng('l1').tensor_add(T4[:, :, 0:nt], V[:, 0:4, 0:nt], V[:, 4:8, 0:nt])
            eng('l2').tensor_add(T2[:, :, 0:nt], T4[:, 0:2, 0:nt], T4[:, 2:4, 0:nt])
            eng('l3').tensor_add(kf[:, 0:nt], T2[:, 0, 0:nt], T2[:, 1, 0:nt])

            if asn['pipe']:
                pending.append((X, D, kf, n0, nt))
            else:
                emit_out_stage(emit_relu((X, D, kf, n0, nt)))
            n0 += nt

        for item in pending:
            emit_out_stage(emit_relu(item))

    nc.finalize()
    return nc


_CACHED = {}
CFG = (DEFAULT_CHUNKS, ())


def _freeze(cfg):
    chunks, assign = cfg
    return (tuple(chunks), tuple(sorted(dict(assign).items())))


def _get_program(cfg=None):
    cfg = cfg or CFG
    key = _freeze(cfg)
    if key not in _CACHED:
        _CACHED[key] = _build(tuple(cfg[0]), dict(cfg[1]))
    return _CACHED[key]


def _run(x: np.ndarray, W: np.ndarray, trace: bool = False, cfg=None):
    nc = _get_program(cfg)
    w2t = np.zeros((128, 128), dtype=np.float16)
    wt = np.ascontiguousarray(W.T).astype(np.float16)
    w2t[:64, :64] = wt
    w2t[64:, 64:] = wt
    x16 = np.asarray(x, dtype=np.float16)
    in_maps = [
        {"x": np.ascontiguousarray(x16[c * B_PER_CORE:(c + 1) * B_PER_CORE]),
         "w2t": w2t}
        for c in range(N_CORES)
    ]
    res = run_bass_kernel_spmd(nc, in_maps, list(range(N_CORES)), trace=trace)
    out = np.concatenate([res.results[c]["out"] for c in range(N_CORES)], axis=0)
    return out.astype(np.float32), res


def kernel(x: np.ndarray, W: np.ndarray) -> np.ndarray:
    out, _ = _run(np.asarray(x, dtype=np.float32), np.asarray(W, dtype=np.float32))
    return out


# revision 8
# speedup vs baseline: 1.0650x; 1.0650x over previous
"""Trainium2 Bass kernel for nn_LNKillingRelu (fp16 pipeline).

Math (per batch b, channel g, point n; L=8 lie-algebra coords):
    d[b,g,:,n]  = sum_f W[g,f] * x[b,f,:,n]          (64x64 linear over channels)
    kf[b,g,n]   = x[b,g,:,n]^T K d[b,g,:,n]          (8x8 Killing bilinear form)
    out         = x + relu(kf) * d                    (broadcast kf over L)

K is symmetric & sparse: pairs (0<->6, 1<->7, 2<->3) plus [[2,1],[1,2]] on
coords (4,5).  We use kf = sum_l (Kx)[l] * d[l] with
(Kx) = [x6, x7, x3, x2, 2x4+x5, x4+2x5, x0, x1].

Everything on-device runs in fp16 (tolerance is 2e-2 scale-relative; fp16
keeps it ~1e-3): halves DMA bytes (the binding resource - all DMA transfers
serialize at ~360B/ns aggregate), doubles DVE throughput (2x packed mode),
and runs the PE at 1 cycle/row instead of 4.  Host converts fp32<->fp16.

Sharding: data-parallel over batch B=16 -> 2 batches per core (8 cores).
Per-core layout: partitions = (batch-pair, F=64) = 128; free = (L, n-chunk).
PE computes d with a block-diag(W^T, W^T) 128x128 stationary fp16 weight.
ACT drains PSUM->SBUF (fp32->fp16); DVE+GPSIMD share the Killing products,
tree-reduce, and the out = x + relu(kf)*d tail.

Software pipelining: relu(k-1) and the out-stage(k-1) are emitted inside
chunk k - relu goes on ACT *before* chunk k's PSUM drains (so ACT never
stalls on the DVE kf-chain), and the out-stage fills the DVE/Pool gap while
chunk k's D is still in the PE/ACT pipe.  Chunk sizes ramp 256 -> 512 -> 256
to shorten the pipeline head/tail; all tiles are allocated at max width and
column-sliced so the tile pools see a single tag.
"""

import numpy as np
from contextlib import ExitStack

import concourse.bass as bass
import concourse.bacc as bacc
import concourse.tile as tile
from concourse import mybir
from concourse.bass_utils import run_bass_kernel_spmd

B, F, L, N = 16, 64, 8, 4096
N_CORES = 8
B_PER_CORE = B // N_CORES  # 2

F16 = mybir.dt.float16
F32 = mybir.dt.float32
MULT = mybir.AluOpType.mult
ADD = mybir.AluOpType.add

NQ = 256  # psum quarter width (4 banks fp32 at L=8)


def _bcast_l(ap: bass.AP, l: int) -> bass.AP:
    """[128, NT] AP -> [128, l, NT] with a zero-stride middle dim."""
    return bass.AP(tensor=ap.tensor, offset=ap.offset,
                   ap=[ap.ap[0], [0, l], ap.ap[1]])


# default engine assignment: 'v' = DVE, 'p' = GPSIMD(Pool)
DEFAULT_ASSIGN = dict(pa='v', pb='v', pc='p', pd='v', pe='v',
                      xe0='p', xe1='p', l1='v', l2='v', l3='v',
                      osplit=3, pipe=1)
DEFAULT_CHUNKS = (256, 256, 512, 512, 512, 512, 512, 512, 256, 256)


def _build(chunks=DEFAULT_CHUNKS, assign=None):
    asn = dict(DEFAULT_ASSIGN)
    if assign:
        asn.update(assign)
    assert sum(chunks) == N
    ntmax = max(chunks)
    nc = bacc.Bacc("TRN2", target_bir_lowering=False, debug=False,
                   num_devices=N_CORES)
    x = nc.dram_tensor("x", [B_PER_CORE, F, L, N], F16, kind="ExternalInput").ap()
    w2t = nc.dram_tensor("w2t", [128, 128], F16, kind="ExternalInput").ap()
    out = nc.dram_tensor("out", [B_PER_CORE, F, L, N], F16, kind="ExternalOutput").ap()

    xv = x.rearrange("b f l n -> (b f) l n")    # [128, 8, N]
    ov = out.rearrange("b f l n -> (b f) l n")

    def eng(key):
        return nc.vector if asn[key] == 'v' else nc.gpsimd

    with ExitStack() as ctx:
        tc = ctx.enter_context(tile.TileContext(nc))
        singles = ctx.enter_context(tc.tile_pool(name="singles", bufs=1))
        xpool = ctx.enter_context(tc.tile_pool(name="xp", bufs=3))
        dpool = ctx.enter_context(tc.tile_pool(name="dp", bufs=3))
        vpool = ctx.enter_context(tc.tile_pool(name="vp", bufs=2))
        tpool = ctx.enter_context(tc.tile_pool(name="tp", bufs=2))
        mpool = ctx.enter_context(tc.tile_pool(name="mp", bufs=2))
        opool = ctx.enter_context(tc.tile_pool(name="op", bufs=3))
        rpool = ctx.enter_context(tc.tile_pool(name="rp", bufs=3))
        psum = ctx.enter_context(tc.tile_pool(name="ps", bufs=2, space="PSUM"))

        w_sb = singles.tile([128, 128], F16)
        nc.sync.dma_start(out=w_sb[:], in_=w2t)

        pending = []  # deferred tail: (X, D, kf, n0, nt)

        def emit_relu(item):
            X, D, kf, n0, nt = item
            r = rpool.tile([128, ntmax], F16, tag="r")
            nc.scalar.activation(r[:, 0:nt], kf[:, 0:nt],
                                 mybir.ActivationFunctionType.Relu)
            return (X, D, r, n0, nt)

        def emit_out_stage(item):
            X, D, r, n0, nt = item
            M = mpool.tile([128, L, ntmax], F16, tag="M")
            O = opool.tile([128, L, ntmax], F16, tag="O")
            s = asn['osplit']
            if s > 0:
                nc.vector.tensor_mul(M[:, 0:s, 0:nt], D[:, 0:s, 0:nt],
                                     _bcast_l(r[:, 0:nt], s))
                nc.vector.tensor_add(O[:, 0:s, 0:nt], X[:, 0:s, 0:nt],
                                     M[:, 0:s, 0:nt])
            if s < L:
                rb = _bcast_l(r[:, 0:nt], L - s)
                nc.gpsimd.tensor_mul(M[:, s:L, 0:nt], D[:, s:L, 0:nt], rb)
                nc.gpsimd.tensor_add(O[:, s:L, 0:nt], X[:, s:L, 0:nt],
                                     M[:, s:L, 0:nt])
            nc.sync.dma_start(out=ov[:, :, n0:n0 + nt], in_=O[:, :, 0:nt])

        n0 = 0
        for nt in chunks:
            # in-DMA split by column halves: the first half's matmuls can
            # start while the second half is still transferring.
            X = xpool.tile([128, L, ntmax], F16, tag="X")
            nh = nt // 2
            if nh >= NQ:  # below 256 cols the 512B-descriptor rate halves
                nc.sync.dma_start(out=X[:, :, 0:nh], in_=xv[:, :, n0:n0 + nh])
                nc.sync.dma_start(out=X[:, :, nh:nt], in_=xv[:, :, n0 + nh:n0 + nt])
            else:
                nc.sync.dma_start(out=X[:, :, 0:nt], in_=xv[:, :, n0:n0 + nt])

            # Xe = (2x4+x5, x4+2x5): only needs X, fills Pool early.
            Xe = tpool.tile([128, 2, ntmax], F16, tag="Xe")
            eng('xe0').scalar_tensor_tensor(Xe[:, 0, 0:nt], in0=X[:, 4, 0:nt],
                                            scalar=2.0, in1=X[:, 5, 0:nt],
                                            op0=MULT, op1=ADD)
            eng('xe1').scalar_tensor_tensor(Xe[:, 1, 0:nt], in0=X[:, 5, 0:nt],
                                            scalar=2.0, in1=X[:, 4, 0:nt],
                                            op0=MULT, op1=ADD)

            # relu of the previous chunk: on ACT *before* this chunk's
            # copies, so it never blocks them (its input is long done).
            prev = None
            if pending:
                prev = emit_relu(pending.pop(0))

            # D = W @ X via psum quarter-tiles (4 banks each, double buffered)
            D = dpool.tile([128, L, ntmax], F16, tag="D")
            for q in range(nt // NQ):
                c0 = q * NQ
                ps = psum.tile([128, L, NQ], F32, tag="ps")
                for k in range(L // 2):
                    sl = slice(2 * k, 2 * k + 2)
                    nc.tensor.matmul(ps[:, sl, :], lhsT=w_sb[:],
                                     rhs=X[:, sl, c0:c0 + NQ],
                                     start=True, stop=True)
                nc.scalar.copy(D[:, :, c0:c0 + NQ], ps[:])

            # deferred out-stage of the previous chunk: ready to run, fills
            # the DVE/Pool gap while this chunk's D is in the PE/ACT pipe.
            if prev is not None:
                emit_out_stage(prev)

            # V[l] = (Kx)[l] * D[l]
            V = vpool.tile([128, L, ntmax], F16, tag="V")
            eng('pa').tensor_mul(V[:, 0:2, 0:nt], X[:, 6:8, 0:nt], D[:, 0:2, 0:nt])
            eng('pb').tensor_mul(V[:, 2, 0:nt], X[:, 3, 0:nt], D[:, 2, 0:nt])
            eng('pc').tensor_mul(V[:, 3, 0:nt], X[:, 2, 0:nt], D[:, 3, 0:nt])
            eng('pd').tensor_mul(V[:, 4:6, 0:nt], Xe[:, :, 0:nt], D[:, 4:6, 0:nt])
            eng('pe').tensor_mul(V[:, 6:8, 0:nt], X[:, 0:2, 0:nt], D[:, 6:8, 0:nt])

            # kf = sum_l V[l] via tree adds (tensor_tensor keeps the fp16 2x
            # DVE mode; tensor_reduce over strided l would run at full rate)
            T4 = tpool.tile([128, 4, ntmax], F16, tag="T4")
            T2 = tpool.tile([128, 2, ntmax], F16, tag="T2")
            kf = rpool.tile([128, ntmax], F16, tag="kf")
            eng('l1').tensor_add(T4[:, :, 0:nt], V[:, 0:4, 0:nt], V[:, 4:8, 0:nt])
            eng('l2').tensor_add(T2[:, :, 0:nt], T4[:, 0:2, 0:nt], T4[:, 2:4, 0:nt])
            eng('l3').tensor_add(kf[:, 0:nt], T2[:, 0, 0:nt], T2[:, 1, 0:nt])

            if asn['pipe']:
                pending.append((X, D, kf, n0, nt))
            else:
                emit_out_stage(emit_relu((X, D, kf, n0, nt)))
            n0 += nt

        for item in pending:
            emit_out_stage(emit_relu(item))

    nc.finalize()
    return nc


_CACHED = {}
CFG = (DEFAULT_CHUNKS, ())


def _freeze(cfg):
    chunks, assign = cfg
    return (tuple(chunks), tuple(sorted(dict(assign).items())))


def _get_program(cfg=None):
    cfg = cfg or CFG
    key = _freeze(cfg)
    if key not in _CACHED:
        _CACHED[key] = _build(tuple(cfg[0]), dict(cfg[1]))
    return _CACHED[key]


def _run(x: np.ndarray, W: np.ndarray, trace: bool = False, cfg=None):
    nc = _get_program(cfg)
    w2t = np.zeros((128, 128), dtype=np.float16)
    wt = np.ascontiguousarray(W.T).astype(np.float16)
    w2t[:64, :64] = wt
    w2t[64:, 64:] = wt
    x16 = np.asarray(x, dtype=np.float16)
    in_maps = [
        {"x": np.ascontiguousarray(x16[c * B_PER_CORE:(c + 1) * B_PER_CORE]),
         "w2t": w2t}
        for c in range(N_CORES)
    ]
    res = run_bass_kernel_spmd(nc, in_maps, list(range(N_CORES)), trace=trace)
    out = np.concatenate([res.results[c]["out"] for c in range(N_CORES)], axis=0)
    return out.astype(np.float32), res


def kernel(x: np.ndarray, W: np.ndarray) -> np.ndarray:
    out, _ = _run(np.asarray(x, dtype=np.float32), np.asarray(W, dtype=np.float32))
    return out


# revision 13
# speedup vs baseline: 1.1193x; 1.0510x over previous
"""Trainium2 Bass kernel for nn_LNKillingRelu (fp16 pipeline).

Math (per batch b, channel g, point n; L=8 lie-algebra coords):
    d[b,g,:,n]  = sum_f W[g,f] * x[b,f,:,n]          (64x64 linear over channels)
    kf[b,g,n]   = x[b,g,:,n]^T K d[b,g,:,n]          (8x8 Killing bilinear form)
    out         = x + relu(kf) * d                    (broadcast kf over L)

K is symmetric & sparse: pairs (0<->6, 1<->7, 2<->3) plus [[2,1],[1,2]] on
coords (4,5).  We use kf = sum_l (Kx)[l] * d[l] with
(Kx) = [x6, x7, x3, x2, 2x4+x5, x4+2x5, x0, x1].

Everything on-device runs in fp16 (tolerance is 2e-2 scale-relative; fp16
keeps it ~1e-3): halves DMA bytes (the binding resource - all DMA transfers
serialize at ~360B/ns aggregate), doubles DVE throughput (2x packed mode),
and runs the PE at 1 cycle/row instead of 4.  Host converts fp32<->fp16.

Sharding: data-parallel over batch B=16 -> 2 batches per core (8 cores).
Per-core layout: partitions = (batch-pair, F=64) = 128; free = (L, n-chunk).
PE computes d with a block-diag(W^T, W^T) 128x128 stationary fp16 weight.
ACT drains PSUM->SBUF (fp32->fp16); DVE+GPSIMD share the Killing products,
tree-reduce, and the out = x + relu(kf)*d tail.

Software pipelining: relu(k-1) and the out-stage(k-1) are emitted inside
chunk k - relu goes on ACT *before* chunk k's PSUM drains (so ACT never
stalls on the DVE kf-chain), and the out-stage fills the DVE/Pool gap while
chunk k's D is still in the PE/ACT pipe.  Chunk sizes ramp 256 -> 512 -> 256
to shorten the pipeline head/tail; all tiles are allocated at max width and
column-sliced so the tile pools see a single tag.
"""

import numpy as np
from contextlib import ExitStack

import concourse.bass as bass
import concourse.bacc as bacc
import concourse.tile as tile
from concourse import mybir
from concourse.bass_utils import run_bass_kernel_spmd

B, F, L, N = 16, 64, 8, 4096
N_CORES = 8
B_PER_CORE = B // N_CORES  # 2

F16 = mybir.dt.float16
F32 = mybir.dt.float32
MULT = mybir.AluOpType.mult
ADD = mybir.AluOpType.add

NQ = 256  # psum quarter width (4 banks fp32 at L=8)


def _bcast_l(ap: bass.AP, l: int) -> bass.AP:
    """[128, NT] AP -> [128, l, NT] with a zero-stride middle dim."""
    return bass.AP(tensor=ap.tensor, offset=ap.offset,
                   ap=[ap.ap[0], [0, l], ap.ap[1]])


# default engine assignment: 'v' = DVE, 'p' = GPSIMD(Pool)
DEFAULT_ASSIGN = dict(pa='v', pb='v', pc='p', pd='v', pe='v',
                      xe0='p', xe1='p', l1='v', l2='v', l3='v',
                      osplit=3, pipe=1, ocolsplit=1)
DEFAULT_CHUNKS = (256, 256, 512, 512, 512, 512, 512, 512, 256, 256)


def _build(chunks=DEFAULT_CHUNKS, assign=None):
    asn = dict(DEFAULT_ASSIGN)
    if assign:
        asn.update(assign)
    assert sum(chunks) == N
    ntmax = max(chunks)
    nc = bacc.Bacc("TRN2", target_bir_lowering=False, debug=False,
                   num_devices=N_CORES)
    x = nc.dram_tensor("x", [B_PER_CORE, F, L, N], F16, kind="ExternalInput").ap()
    w2t = nc.dram_tensor("w2t", [128, 128], F16, kind="ExternalInput").ap()
    out = nc.dram_tensor("out", [B_PER_CORE, F, L, N], F16, kind="ExternalOutput").ap()

    xv = x.rearrange("b f l n -> (b f) l n")    # [128, 8, N]
    ov = out.rearrange("b f l n -> (b f) l n")

    def eng(key):
        return nc.vector if asn[key] == 'v' else nc.gpsimd

    with ExitStack() as ctx:
        tc = ctx.enter_context(tile.TileContext(nc))
        singles = ctx.enter_context(tc.tile_pool(name="singles", bufs=1))
        xpool = ctx.enter_context(tc.tile_pool(name="xp", bufs=3))
        dpool = ctx.enter_context(tc.tile_pool(name="dp", bufs=3))
        vpool = ctx.enter_context(tc.tile_pool(name="vp", bufs=2))
        tpool = ctx.enter_context(tc.tile_pool(name="tp", bufs=2))
        mpool = ctx.enter_context(tc.tile_pool(name="mp", bufs=2))
        opool = ctx.enter_context(tc.tile_pool(name="op", bufs=3))
        rpool = ctx.enter_context(tc.tile_pool(name="rp", bufs=3))
        psum = ctx.enter_context(tc.tile_pool(name="ps", bufs=4, space="PSUM"))

        w_sb = singles.tile([128, 128], F16)
        nc.sync.dma_start(out=w_sb[:], in_=w2t)

        pending = []  # deferred tail: (X, D, kf, n0, nt)

        def emit_relu(item):
            X, D, kf, n0, nt = item
            r = rpool.tile([128, ntmax], F16, tag="r")
            nc.scalar.activation(r[:, 0:nt], kf[:, 0:nt],
                                 mybir.ActivationFunctionType.Relu)
            return (X, D, r, n0, nt)

        def emit_out_stage(item):
            X, D, r, n0, nt = item
            M = mpool.tile([128, L, ntmax], F16, tag="M")
            O = opool.tile([128, L, ntmax], F16, tag="O")
            s = asn['osplit']
            nh = nt // 2
            # Pool's share (and the out-DMA) go in column halves so the first
            # half can enter the serialized DMA queue while the second is
            # still being computed.  DVE's share is one op (it finishes early).
            colsplit = asn['ocolsplit'] and s < L and nh >= NQ
            if s > 0:
                nc.vector.tensor_mul(M[:, 0:s, 0:nt], D[:, 0:s, 0:nt],
                                     _bcast_l(r[:, 0:nt], s))
                nc.vector.tensor_add(O[:, 0:s, 0:nt], X[:, 0:s, 0:nt],
                                     M[:, 0:s, 0:nt])
            pieces = [(0, nh), (nh, nt)] if colsplit else [(0, nt)]
            for (a, b) in pieces:
                if s < L:
                    rb = _bcast_l(r[:, a:b], L - s)
                    nc.gpsimd.tensor_mul(M[:, s:L, a:b], D[:, s:L, a:b], rb)
                    nc.gpsimd.tensor_add(O[:, s:L, a:b], X[:, s:L, a:b],
                                         M[:, s:L, a:b])
                nc.sync.dma_start(out=ov[:, :, n0 + a:n0 + b], in_=O[:, :, a:b])

        n0 = 0
        for nt in chunks:
            # in-DMA split by column halves: the first half's matmuls can
            # start while the second half is still transferring.
            X = xpool.tile([128, L, ntmax], F16, tag="X")
            nh = nt // 2
            if nh >= NQ:  # below 256 cols the 512B-descriptor rate halves
                nc.sync.dma_start(out=X[:, :, 0:nh], in_=xv[:, :, n0:n0 + nh])
                nc.sync.dma_start(out=X[:, :, nh:nt], in_=xv[:, :, n0 + nh:n0 + nt])
            else:
                nc.sync.dma_start(out=X[:, :, 0:nt], in_=xv[:, :, n0:n0 + nt])

            # Xe = (2x4+x5, x4+2x5): only needs X, fills Pool early.
            Xe = tpool.tile([128, 2, ntmax], F16, tag="Xe")
            eng('xe0').scalar_tensor_tensor(Xe[:, 0, 0:nt], in0=X[:, 4, 0:nt],
                                            scalar=2.0, in1=X[:, 5, 0:nt],
                                            op0=MULT, op1=ADD)
            eng('xe1').scalar_tensor_tensor(Xe[:, 1, 0:nt], in0=X[:, 5, 0:nt],
                                            scalar=2.0, in1=X[:, 4, 0:nt],
                                            op0=MULT, op1=ADD)

            # relu of the previous chunk: on ACT *before* this chunk's
            # copies, so it never blocks them (its input is long done).
            prev = None
            if pending:
                prev = emit_relu(pending.pop(0))

            # D = W @ X at l-pair granularity: psum tiles are [128,2,ntmax]
            # (2 banks, 4 in flight), each drained by ACT right after its two
            # matmuls - downstream products fire as soon as their pair lands.
            D = dpool.tile([128, L, ntmax], F16, tag="D")
            for p in range(L // 2):
                ps = psum.tile([128, 2, ntmax], F32, tag="ps")
                for k in range(2):
                    l = 2 * p + k
                    nc.tensor.matmul(ps[:, k, 0:nt], lhsT=w_sb[:],
                                     rhs=X[:, l, 0:nt],
                                     start=True, stop=True)
                nc.scalar.copy(D[:, 2 * p:2 * p + 2, 0:nt], ps[:, :, 0:nt])

            # deferred out-stage of the previous chunk: ready to run, fills
            # the DVE/Pool gap while this chunk's D is in the PE/ACT pipe.
            if prev is not None:
                emit_out_stage(prev)

            # V[l] = (Kx)[l] * D[l]
            V = vpool.tile([128, L, ntmax], F16, tag="V")
            eng('pa').tensor_mul(V[:, 0:2, 0:nt], X[:, 6:8, 0:nt], D[:, 0:2, 0:nt])
            eng('pb').tensor_mul(V[:, 2, 0:nt], X[:, 3, 0:nt], D[:, 2, 0:nt])
            eng('pc').tensor_mul(V[:, 3, 0:nt], X[:, 2, 0:nt], D[:, 3, 0:nt])
            eng('pd').tensor_mul(V[:, 4:6, 0:nt], Xe[:, :, 0:nt], D[:, 4:6, 0:nt])
            eng('pe').tensor_mul(V[:, 6:8, 0:nt], X[:, 0:2, 0:nt], D[:, 6:8, 0:nt])

            # kf = sum_l V[l] via tree adds (tensor_tensor keeps the fp16 2x
            # DVE mode; tensor_reduce over strided l would run at full rate)
            T4 = tpool.tile([128, 4, ntmax], F16, tag="T4")
            T2 = tpool.tile([128, 2, ntmax], F16, tag="T2")
            kf = rpool.tile([128, ntmax], F16, tag="kf")
            eng('l1').tensor_add(T4[:, :, 0:nt], V[:, 0:4, 0:nt], V[:, 4:8, 0:nt])
            eng('l2').tensor_add(T2[:, :, 0:nt], T4[:, 0:2, 0:nt], T4[:, 2:4, 0:nt])
            eng('l3').tensor_add(kf[:, 0:nt], T2[:, 0, 0:nt], T2[:, 1, 0:nt])

            if asn['pipe']:
                pending.append((X, D, kf, n0, nt))
            else:
                emit_out_stage(emit_relu((X, D, kf, n0, nt)))
            n0 += nt

        for item in pending:
            emit_out_stage(emit_relu(item))

    nc.finalize()
    return nc


_CACHED = {}
CFG = (DEFAULT_CHUNKS, ())


def _freeze(cfg):
    chunks, assign = cfg
    return (tuple(chunks), tuple(sorted(dict(assign).items())))


def _get_program(cfg=None):
    cfg = cfg or CFG
    key = _freeze(cfg)
    if key not in _CACHED:
        _CACHED[key] = _build(tuple(cfg[0]), dict(cfg[1]))
    return _CACHED[key]


def _run(x: np.ndarray, W: np.ndarray, trace: bool = False, cfg=None):
    nc = _get_program(cfg)
    w2t = np.zeros((128, 128), dtype=np.float16)
    wt = np.ascontiguousarray(W.T).astype(np.float16)
    w2t[:64, :64] = wt
    w2t[64:, 64:] = wt
    x16 = np.asarray(x, dtype=np.float16)
    in_maps = [
        {"x": np.ascontiguousarray(x16[c * B_PER_CORE:(c + 1) * B_PER_CORE]),
         "w2t": w2t}
        for c in range(N_CORES)
    ]
    res = run_bass_kernel_spmd(nc, in_maps, list(range(N_CORES)), trace=trace)
    out = np.concatenate([res.results[c]["out"] for c in range(N_CORES)], axis=0)
    return out.astype(np.float32), res


def kernel(x: np.ndarray, W: np.ndarray) -> np.ndarray:
    out, _ = _run(np.asarray(x, dtype=np.float32), np.asarray(W, dtype=np.float32))
    return out
